# revision 2
# baseline (speedup 1.0000x reference)
"""CategorySpecificLinear TRN2 kernel.

out[b] = x[b] @ W[cat_ids[b]] + bias[cat_ids[b]]
  x: [64, 512, 1024] f32, W: [32, 1024, 4096] f32, b: [32, 4096] f32 -> out [64, 512, 4096] f32

Strategy: data-parallel over batch (8 batches per core on 8 cores). The
category gather, fp16 conversion, x transpose, and SBUF tile layout are all
done on the host (cat_ids are known at launch), so each core receives its 8
per-batch weight matrices pre-tiled for fully contiguous DMA. Matmuls run in
fp16 (fp32 PSUM accumulation): same PE throughput as bf16 on TRN2 but ~8x
better accuracy (~3e-4 rel), 4x faster than native fp32.
"""
import numpy as np

B_TOTAL = 64
N_CORES = 8
B = B_TOTAL // N_CORES  # batches per core
S = 512    # seq
K = 1024   # input_dim
H = 4096   # hidden_dim
P = 128
KT = K // P   # 8 k-tiles
MT = S // P   # 4 m-tiles
NW = 512      # hidden tile width (one PSUM bank of f32)
NT = H // NW  # 8 n-tiles

_NC = None


def _build_nc():
    global _NC
    if _NC is not None:
        return _NC

    import concourse.mybir as mybir
    import concourse.tile as tile
    from concourse import bacc

    f16 = mybir.dt.float16
    f32 = mybir.dt.float32

    nc = bacc.Bacc("TRN2", target_bir_lowering=False, debug=False, num_devices=N_CORES)
    # Host-pre-tiled layouts: per-(b,n) weight tile [P, KT, NW] contiguous,
    # per-b x tile [P, KT, S] contiguous -> per-partition 8 KiB DMA bursts.
    xt = nc.dram_tensor("xt", [B, P, KT, S], f16, kind="ExternalInput").ap()
    w = nc.dram_tensor("w", [B, NT, P, KT, NW], f16, kind="ExternalInput").ap()
    bias = nc.dram_tensor("bias", [B, H], f32, kind="ExternalInput").ap()
    out = nc.dram_tensor("out", [B, S, H], f32, kind="ExternalOutput").ap()

    with tile.TileContext(nc) as tc:
        with (
            tc.tile_pool(name="xtp", bufs=2) as xtp,
            tc.tile_pool(name="wp", bufs=6) as wp,
            tc.tile_pool(name="bp", bufs=2) as bp,
            tc.tile_pool(name="op", bufs=4) as op,
            tc.tile_pool(name="ps", bufs=4, space="PSUM") as ps,
        ):
            for b_i in range(B):
                xt_sb = xtp.tile([P, KT, S], f16, tag="xt")
                nc.sync.dma_start(xt_sb[:, 0:4, :], xt[b_i, :, 0:4, :])
                nc.sync.dma_start(xt_sb[:, 4:8, :], xt[b_i, :, 4:8, :])
                bias_row = bp.tile([1, H], f32, tag="bias_row")
                nc.sync.dma_start(bias_row[:], bias[b_i][None, :])
                bias_bc = bp.tile([P, H], f32, tag="bias_bc")
                nc.gpsimd.partition_broadcast(bias_bc[:], bias_row[:])
                for n_i in range(NT):
                    w_sb = wp.tile([P, KT, NW], f16, tag="w")
                    nc.sync.dma_start(w_sb[:, 0:4, :], w[b_i, n_i, :, 0:4, :])
                    nc.sync.dma_start(w_sb[:, 4:8, :], w[b_i, n_i, :, 4:8, :])
                    for m_i in range(MT):
                        pt = ps.tile([P, NW], f32, tag="psum")
                        for k_i in range(KT):
                            nc.tensor.matmul(
                                pt[:],
                                xt_sb[:, k_i, m_i * P : (m_i + 1) * P],
                                w_sb[:, k_i, :],
                                start=(k_i == 0),
                                stop=(k_i == KT - 1),
                            )
                        ot = op.tile([P, NW], f32, tag="out")
                        nc.vector.tensor_add(
                            ot[:], pt[:], bias_bc[:, n_i * NW : (n_i + 1) * NW]
                        )
                        # separate HWDGE queue (scalar) so output bursts don't
                        # head-of-line-block the weight loads on the sync queue
                        nc.scalar.dma_start(
                            out[b_i, m_i * P : (m_i + 1) * P, n_i * NW : (n_i + 1) * NW],
                            ot[:],
                        )
    nc.compile()
    _NC = nc
    return nc


def _prep_in_maps(x, cat_ids, W, b):
    W16 = W.astype(np.float16)                      # [32, K, H]
    Wg = W16[cat_ids]                               # [64, K, H]
    # tile layout [B_TOTAL, NT, P, KT, NW]
    Wt = np.ascontiguousarray(
        Wg.reshape(B_TOTAL, KT, P, NT, NW).transpose(0, 3, 2, 1, 4)
    )
    x16 = x.astype(np.float16)                      # [64, S, K]
    xt16 = x16.transpose(0, 2, 1)                   # [64, K, S] (view)
    # tile layout [B_TOTAL, P, KT, S]
    xtt = np.ascontiguousarray(
        np.ascontiguousarray(xt16).reshape(B_TOTAL, KT, P, S).transpose(0, 2, 1, 3)
    )
    bg = b[cat_ids].astype(np.float32)              # [64, H]

    in_maps = []
    for c in range(N_CORES):
        sl = slice(B * c, B * (c + 1))
        in_maps.append(
            {
                "xt": np.ascontiguousarray(xtt[sl]),
                "w": np.ascontiguousarray(Wt[sl]),
                "bias": np.ascontiguousarray(bg[sl]),
            }
        )
    return in_maps


def kernel(x, cat_ids, W, b):
    from concourse.bass_utils import run_bass_kernel_spmd

    x = np.asarray(x)
    cat_ids = np.asarray(cat_ids).astype(np.int64)
    W = np.asarray(W)
    b = np.asarray(b)

    nc = _build_nc()
    in_maps = _prep_in_maps(x, cat_ids, W, b)
    res = run_bass_kernel_spmd(nc, in_maps, core_ids=list(range(N_CORES)))
    out = np.concatenate([r["out"] for r in res.results], axis=0)
    return out.astype(np.float32, copy=False)


# revision 3
# speedup vs baseline: 1.0803x; 1.0803x over previous
"""CategorySpecificLinear TRN2 kernel.

out[b] = x[b] @ W[cat_ids[b]] + bias[cat_ids[b]]
  x: [64, 512, 1024] f32, W: [32, 1024, 4096] f32, b: [32, 4096] f32
  -> out [64, 512, 4096] f32

Strategy: data-parallel over batch — 8 batches per core on 8 NeuronCores.
The category gather, fp16 conversion, and x transpose are done on the host
(cat_ids are known at launch), so each core receives its 8 per-batch weight
matrices directly; no on-device indexing is needed. Matmuls run in fp16 with
fp32 PSUM accumulation: same PE throughput as bf16 on TRN2 (1 cycle/row) but
~8x better accuracy (~3e-4 rel), and 4x faster than native fp32 (4 cycles/row).

Per core: 2048 matmuls of [128k,128m]@[128k,512n] at the warm issue-rate
floor (~216 ns each). Weight loads ride the sync HWDGE queue; output writes
go through the scalar HWDGE queue so they cannot head-of-line-block the
weight stream (worth ~50 us). Measured ~465 us HW exec time.
"""
import numpy as np

B_TOTAL = 64
N_CORES = 8
B = B_TOTAL // N_CORES  # batches per core
S = 512    # seq
K = 1024   # input_dim
H = 4096   # hidden_dim
P = 128
KT = K // P   # 8 k-tiles
MT = S // P   # 4 m-tiles
NW = 512      # hidden tile width (one fp32 PSUM bank)
NT = H // NW  # 8 n-tiles

_NC = None


def _build_nc():
    global _NC
    if _NC is not None:
        return _NC

    import concourse.mybir as mybir
    import concourse.tile as tile
    from concourse import bacc

    f16 = mybir.dt.float16
    f32 = mybir.dt.float32

    nc = bacc.Bacc("TRN2", target_bir_lowering=False, debug=False, num_devices=N_CORES)
    xt = nc.dram_tensor("xt", [B, K, S], f16, kind="ExternalInput").ap()
    w = nc.dram_tensor("w", [B, K, H], f16, kind="ExternalInput").ap()
    bias = nc.dram_tensor("bias", [B, H], f32, kind="ExternalInput").ap()
    out = nc.dram_tensor("out", [B, S, H], f32, kind="ExternalOutput").ap()

    with tile.TileContext(nc) as tc:
        with (
            tc.tile_pool(name="xtp", bufs=2) as xtp,
            tc.tile_pool(name="wp", bufs=6) as wp,
            tc.tile_pool(name="bp", bufs=2) as bp,
            tc.tile_pool(name="op", bufs=6) as op,
            tc.tile_pool(name="ps", bufs=6, space="PSUM") as ps,
        ):
            for b_i in range(B):
                xt_sb = xtp.tile([P, KT, S], f16, tag="xt")
                for sp in range(2):
                    k0, k1 = sp * (KT // 2), (sp + 1) * (KT // 2)
                    nc.sync.dma_start(
                        xt_sb[:, k0:k1, :],
                        xt[b_i, k0 * P : k1 * P, :].rearrange("(ko p) s -> p ko s", p=P),
                    )
                bias_row = bp.tile([1, H], f32, tag="bias_row")
                nc.sync.dma_start(bias_row[:], bias[b_i][None, :])
                bias_bc = bp.tile([P, H], f32, tag="bias_bc")
                nc.gpsimd.partition_broadcast(bias_bc[:], bias_row[:])
                for n_i in range(NT):
                    w_sb = wp.tile([P, KT, NW], f16, tag="w")
                    for sp in range(2):
                        k0, k1 = sp * (KT // 2), (sp + 1) * (KT // 2)
                        nc.sync.dma_start(
                            w_sb[:, k0:k1, :],
                            w[b_i, k0 * P : k1 * P, n_i * NW : (n_i + 1) * NW].rearrange(
                                "(ko p) n -> p ko n", p=P
                            ),
                        )
                    for m_i in range(MT):
                        pt = ps.tile([P, NW], f32, tag="psum")
                        for k_i in range(KT):
                            nc.tensor.matmul(
                                pt[:],
                                xt_sb[:, k_i, m_i * P : (m_i + 1) * P],
                                w_sb[:, k_i, :],
                                start=(k_i == 0),
                                stop=(k_i == KT - 1),
                            )
                        ot = op.tile([P, NW], f32, tag="out")
                        nc.vector.tensor_add(
                            ot[:], pt[:], bias_bc[:, n_i * NW : (n_i + 1) * NW]
                        )
                        # separate HWDGE queue (scalar) so output bursts don't
                        # head-of-line-block the weight loads on the sync queue
                        nc.scalar.dma_start(
                            out[b_i, m_i * P : (m_i + 1) * P, n_i * NW : (n_i + 1) * NW],
                            ot[:],
                        )
    nc.compile()
    _NC = nc
    return nc


def _prep_in_maps(x, cat_ids, W, b):
    W16 = W.astype(np.float16)                      # [32, K, H]
    Wg = W16[cat_ids]                               # [64, K, H]
    x16 = x.astype(np.float16)                      # [64, S, K]
    xt16 = np.ascontiguousarray(x16.transpose(0, 2, 1))  # [64, K, S]
    bg = b[cat_ids].astype(np.float32)              # [64, H]

    in_maps = []
    for c in range(N_CORES):
        sl = slice(B * c, B * (c + 1))
        in_maps.append(
            {
                "xt": np.ascontiguousarray(xt16[sl]),
                "w": np.ascontiguousarray(Wg[sl]),
                "bias": np.ascontiguousarray(bg[sl]),
            }
        )
    return in_maps


def kernel(x, cat_ids, W, b):
    from concourse.bass_utils import run_bass_kernel_spmd

    x = np.asarray(x, dtype=np.float32)
    cat_ids = np.asarray(cat_ids).astype(np.int64)
    W = np.asarray(W, dtype=np.float32)
    b = np.asarray(b, dtype=np.float32)

    nc = _build_nc()
    in_maps = _prep_in_maps(x, cat_ids, W, b)
    res = run_bass_kernel_spmd(nc, in_maps, core_ids=list(range(N_CORES)))
    out = np.concatenate([r["out"] for r in res.results], axis=0)
    return out.astype(np.float32, copy=False)
